# revision 1
# baseline (speedup 1.0000x reference)
"""DCTChannelBlock Trainium2 kernel.

Full computation per sample (b, c, l = 32, 512, 1024):
    freq = DCT-II over last dim  (= x @ D.T, D[k,n] = 2*cos(pi*k*(2n+1)/(2L)))
    h    = LayerNorm_L(freq) * ln_w + ln_b
    h2   = relu(w1 @ h)          # 1x1 conv c -> 2c
    gate = sigmoid(w2 @ h2)      # 1x1 conv 2c -> c
    out  = x * gate

Sharding: data-parallel over batch across 8 NeuronCores (4 samples/core),
weights replicated.

DCT uses the cosine half-sample symmetry D[k, L-1-n] = (-1)^k D[k, n]:
    freq[2k'] = sum_{n<L/2} (x_n + x_{L-1-n}) D[2k', n]
    freq[2k'+1] = sum_{n<L/2} (x_n - x_{L-1-n}) D[2k'+1, n]
which halves the DCT matmul FLOPs. Matmuls run in float32r (full-rate
fp32 mode on the PE, ~1e-4 relative rounding); everything else is fp32.
"""

import contextlib

import numpy as np

import concourse.bass as bass
import concourse.mybir as mybir
import concourse.tile as tile
from concourse import bacc
from concourse.bass_utils import run_bass_kernel_spmd
from concourse.masks import make_identity

B, C, L = 32, 512, 1024
NCORES = 8
BPC = B // NCORES          # samples per core
P = 128                    # partitions
CCH = C // P               # 4 c-chunks
H = L // 2                 # 512, half length for even/odd DCT
JCH = H // P               # 4 n'-chunks (DCT contraction)
OCH = (2 * C) // P         # 8 o-chunks (hidden dim)
KT = 512                   # matmul moving free-dim / PSUM bank
F32 = mybir.dt.float32
F32R = mybir.dt.float32r
EPS = 1e-6


def _build(apply_ln: bool, reps: int = 1, loop_reps: int = 1):
    nc = bacc.Bacc("TRN2", target_bir_lowering=False, debug=False,
                   num_devices=NCORES)
    x_d = nc.dram_tensor("x", [BPC, C, L], F32, kind="ExternalInput")
    dte_d = nc.dram_tensor("dte", [H, H], F32R, kind="ExternalInput")
    dto_d = nc.dram_tensor("dto", [H, H], F32R, kind="ExternalInput")
    w1t_d = nc.dram_tensor("w1t", [C, 2 * C], F32R, kind="ExternalInput")
    w2t_d = nc.dram_tensor("w2t", [2 * C, C], F32R, kind="ExternalInput")
    if apply_ln:
        lnw_d = nc.dram_tensor("lnw", [L], F32, kind="ExternalInput")
        lnb_d = nc.dram_tensor("lnb", [L], F32, kind="ExternalInput")
    out_d = nc.dram_tensor("out", [BPC, C, L], F32, kind="ExternalOutput")

    AF = mybir.ActivationFunctionType
    Alu = mybir.AluOpType

    with tile.TileContext(nc) as tc:
        with (
            tc.tile_pool(name="const", bufs=1) as const,
            tc.tile_pool(name="xp", bufs=2) as xp,
            tc.tile_pool(name="sgp", bufs=2) as sgp,   # sd + gate (shared)
            tc.tile_pool(name="sdtp", bufs=1) as sdtp,
            tc.tile_pool(name="hp", bufs=1 if apply_ln else 2) as hp,
            tc.tile_pool(name="h2p", bufs=1) as h2p,
            tc.tile_pool(name="sp", bufs=4) as sp,
            tc.tile_pool(name="ppd", bufs=3, space="PSUM") as ppd,
            tc.tile_pool(name="ppm", bufs=4, space="PSUM") as ppm,
            tc.tile_pool(name="ptp", bufs=1, space="PSUM") as ptp,
        ):
            # ---- constants (order matters: first x + DCT weights first) ----
            ident_f = const.tile([P, P], F32)
            make_identity(nc, ident_f)
            ident = const.tile([P, P], F32R)
            nc.vector.tensor_copy(ident, ident_f)
            eps_t = const.tile([P, 1], F32)
            nc.vector.memset(eps_t, EPS)

            x_tiles = {}
            if loop_reps == 1:
                x_tiles[0] = xp.tile([P, CCH, L], F32, tag="x", name="x_sb0")
                for cc in range(CCH):
                    nc.sync.dma_start(
                        x_tiles[0][:, cc, :], x_d[0, cc * P:(cc + 1) * P, :])

            dte_sb = const.tile([P, JCH, H], F32R)
            dto_sb = const.tile([P, JCH, H], F32R)
            for j in range(JCH):
                nc.sync.dma_start(dte_sb[:, j, :], dte_d[j * P:(j + 1) * P, :])
                nc.sync.dma_start(dto_sb[:, j, :], dto_d[j * P:(j + 1) * P, :])

            w1t_sb = const.tile([P, CCH, 2 * C], F32R)
            for cc in range(CCH):
                nc.sync.dma_start(w1t_sb[:, cc, :], w1t_d[cc * P:(cc + 1) * P, :])
            w2t_sb = const.tile([P, OCH, C], F32R)
            for oo in range(OCH):
                nc.sync.dma_start(w2t_sb[:, oo, :], w2t_d[oo * P:(oo + 1) * P, :])
            lnw_sb = lnb_sb = None
            if apply_ln:
                lnw_sb = const.tile([P, L], F32)
                lnb_sb = const.tile([P, L], F32)
                nc.gpsimd.dma_start(
                    lnw_sb, bass.AP(tensor=lnw_d, offset=0, ap=[[0, P], [1, L]]))
                nc.gpsimd.dma_start(
                    lnb_sb, bass.AP(tensor=lnb_d, offset=0, ap=[[0, P], [1, L]]))

            def emit_front(it, b):
                """x load, s/d fold, transposes, DCT + LayerNorm -> h."""
                if it in x_tiles:
                    x_sb = x_tiles.pop(it)
                else:
                    x_sb = xp.tile([P, CCH, L], F32, tag="x", name="x_sb")
                    for cc in range(CCH):
                        nc.sync.dma_start(
                            x_sb[:, cc, :], x_d[b, cc * P:(cc + 1) * P, :])

                # s/d halves (GpSimd steady-state; DVE for the first sample
                # so the PE pipeline fills faster at kernel start)
                eng = nc.vector if it == 0 else nc.gpsimd
                sd_sb = sgp.tile([P, CCH, 2, H], F32R, tag="sdg", name="sd_sb")
                for cc in range(CCH):
                    xc = x_sb[:, cc, :]
                    xrev = xc[:, L - 1::-1][:, 0:H]
                    eng.tensor_add(sd_sb[:, cc, 0, :], xc[:, 0:H], xrev)
                    eng.tensor_sub(sd_sb[:, cc, 1, :], xc[:, 0:H], xrev)

                # transpose s/d -> n' on partitions (PE)
                sdT_sb = sdtp.tile([P, 2, JCH, C], F32R, tag="sdT",
                                   name="sdT_sb")
                for cc in range(CCH):
                    for half in range(2):
                        pst = ptp.tile([P, KT], F32R, tag="pst", name="pst")
                        for j in range(JCH):
                            nc.tensor.transpose(
                                pst[:, j * P:(j + 1) * P],
                                sd_sb[:, cc, half, j * P:(j + 1) * P],
                                ident)
                        nc.vector.tensor_copy(
                            out=sdT_sb[:, half, :, cc * P:(cc + 1) * P],
                            in_=pst.rearrange("p (g q) -> p g q", g=JCH))

                # DCT matmuls + LayerNorm -> h (fp32r, natural k order)
                h_sb = hp.tile([P, CCH, L], F32R, tag="h", name="h_sb")
                for cc in range(CCH):
                    ps_e = ppd.tile([P, KT], F32, tag="dct", name="ps_e")
                    ps_o = ppd.tile([P, KT], F32, tag="dct", name="ps_o")
                    for j in range(JCH):
                        nc.tensor.matmul(
                            ps_e, sdT_sb[:, 0, j, cc * P:(cc + 1) * P],
                            dte_sb[:, j, :],
                            start=(j == 0), stop=(j == JCH - 1))
                        nc.tensor.matmul(
                            ps_o, sdT_sb[:, 1, j, cc * P:(cc + 1) * P],
                            dto_sb[:, j, :],
                            start=(j == 0), stop=(j == JCH - 1))
                    st = sp.tile([P, 2, 6], F32, tag="bnst", name="st")
                    nc.vector.bn_stats(st[:, 0, :], ps_e)
                    nc.vector.bn_stats(st[:, 1, :], ps_o)
                    mv = sp.tile([P, 2], F32, tag="mv", name="mv")
                    nc.vector.bn_aggr(mv, st)
                    rstd = sp.tile([P, 1], F32, tag="rstd", name="rstd")
                    nc.scalar.activation(rstd, mv[:, 1:2], AF.Sqrt, bias=eps_t)
                    nc.vector.reciprocal(rstd, rstd)
                    if not apply_ln:
                        for par, ps in ((0, ps_e), (1, ps_o)):
                            nc.vector.tensor_scalar(
                                out=h_sb[:, cc, par:L:2],
                                in0=ps,
                                scalar1=mv[:, 0:1], scalar2=rstd,
                                op0=Alu.subtract, op1=Alu.mult)
                    else:
                        tmp = sp.tile([P, L], F32, tag="lntmp", name="tmp")
                        for par, ps in ((0, ps_e), (1, ps_o)):
                            nc.vector.tensor_scalar(
                                out=tmp[:, par:L:2],
                                in0=ps,
                                scalar1=mv[:, 0:1], scalar2=rstd,
                                op0=Alu.subtract, op1=Alu.mult)
                        nc.vector.tensor_mul(tmp, tmp, lnw_sb)
                        nc.vector.tensor_add(
                            out=h_sb[:, cc, :], in0=tmp, in1=lnb_sb)
                return x_sb, h_sb

            def emit_back(x_sb, h_sb, b):
                """w1+ReLU, w2+Sigmoid, out = x*gate, store."""
                h2_sb = h2p.tile([P, OCH, L], F32R, tag="h2", name="h2_sb")
                for oo in range(OCH):
                    ps0 = ppm.tile([P, KT], F32, tag="mlp", name="ps_w1a")
                    ps1 = ppm.tile([P, KT], F32, tag="mlp", name="ps_w1b")
                    for cc in range(CCH):
                        w = w1t_sb[:, cc, oo * P:(oo + 1) * P]
                        nc.tensor.matmul(ps0, w, h_sb[:, cc, 0:KT],
                                         start=(cc == 0), stop=(cc == CCH - 1))
                        nc.tensor.matmul(ps1, w, h_sb[:, cc, KT:L],
                                         start=(cc == 0), stop=(cc == CCH - 1))
                    nc.scalar.activation(h2_sb[:, oo, 0:KT], ps0, AF.Relu)
                    nc.scalar.activation(h2_sb[:, oo, KT:L], ps1, AF.Relu)

                g_sb = sgp.tile([P, CCH, L], F32, tag="sdg", name="g_sb")
                for cc in range(CCH):
                    ps0 = ppm.tile([P, KT], F32, tag="mlp", name="ps_w2a")
                    ps1 = ppm.tile([P, KT], F32, tag="mlp", name="ps_w2b")
                    for oo in range(OCH):
                        w = w2t_sb[:, oo, cc * P:(cc + 1) * P]
                        nc.tensor.matmul(ps0, w, h2_sb[:, oo, 0:KT],
                                         start=(oo == 0), stop=(oo == OCH - 1))
                        nc.tensor.matmul(ps1, w, h2_sb[:, oo, KT:L],
                                         start=(oo == 0), stop=(oo == OCH - 1))
                    nc.scalar.activation(g_sb[:, cc, 0:KT], ps0, AF.Sigmoid)
                    nc.scalar.activation(g_sb[:, cc, KT:L], ps1, AF.Sigmoid)
                for cc in range(CCH):
                    nc.gpsimd.tensor_mul(
                        g_sb[:, cc, :], g_sb[:, cc, :], x_sb[:, cc, :])
                nc.sync.dma_start(
                    out_d[b].rearrange("(cc p) l -> p cc l", p=P), g_sb)

            schedule = [b for _ in range(reps) for b in range(BPC)]
            ET = mybir.EngineType
            loop_cm = (tc.For_i(0, loop_reps, 1, name="tloop",
                                hint_engines=(ET.PE, ET.DVE, ET.Activation,
                                              ET.Pool, ET.SP))
                       if loop_reps > 1 else contextlib.nullcontext())
            with loop_cm:
                pend = None
                for it, b in enumerate(schedule):
                    front = emit_front(it, b)
                    if pend is not None:
                        emit_back(*pend)
                    pend = (*front, b)
                emit_back(*pend)

    nc.compile()
    return nc


def _dct_matrix():
    """Mirror the reference's float32 construction of D[k, n]."""
    n = np.arange(L, dtype=np.float32)
    k = np.arange(L, dtype=np.float32)
    ang = (np.float32(np.pi / (2.0 * L)) * k)[:, None] * (
        np.float32(2.0) * n[None, :] + np.float32(1.0))
    return (np.float32(2.0) * np.cos(ang)).astype(np.float32)


def _dct_halves():
    D = _dct_matrix()
    dte = np.ascontiguousarray(D[0::2, 0:H].T)  # [n', k'] even rows
    dto = np.ascontiguousarray(D[1::2, 0:H].T)  # [n', k'] odd rows
    return dte, dto


_CACHE = {}


def _get_nc(apply_ln: bool):
    if apply_ln not in _CACHE:
        _CACHE[apply_ln] = _build(apply_ln)
    return _CACHE[apply_ln]


def kernel(x, w1, w2, ln_w, ln_b):
    x = np.ascontiguousarray(np.asarray(x, dtype=np.float32))
    w1 = np.asarray(w1, dtype=np.float32)
    w2 = np.asarray(w2, dtype=np.float32)
    ln_w = np.asarray(ln_w, dtype=np.float32)
    ln_b = np.asarray(ln_b, dtype=np.float32)
    assert x.shape == (B, C, L)

    dte, dto = _dct_halves()
    w1t = np.ascontiguousarray(w1.T)
    w2t = np.ascontiguousarray(w2.T)
    apply_ln = not (np.all(ln_w == 1.0) and np.all(ln_b == 0.0))
    nc = _get_nc(apply_ln)

    in_maps = []
    for i in range(NCORES):
        m = {
            "x": np.ascontiguousarray(x[i * BPC:(i + 1) * BPC]),
            "dte": dte,
            "dto": dto,
            "w1t": w1t,
            "w2t": w2t,
        }
        if apply_ln:
            m["lnw"] = ln_w
            m["lnb"] = ln_b
        in_maps.append(m)

    res = run_bass_kernel_spmd(nc, in_maps, core_ids=list(range(NCORES)))
    return np.concatenate([res.results[i]["out"] for i in range(NCORES)],
                          axis=0)

